# revision 10
# baseline (speedup 1.0000x reference)
"""Trainium2 Bass kernel for nn_ConvBlock (Chebyshev graph conv + BatchNorm + ReLU).

Sharding: data-parallel over batch (B=8 -> 1 sample per NeuronCore).
Per core: power-basis Chebyshev recursion z_j = L z_{j-1} via batched
dma_gather row gathers (bf16) + PE scatter-matmuls (edge weights folded into
host-built one-hot scatter blocks, SBUF-resident), K-stacked GEMM with
host-folded power-basis weights consuming z^T via DMA-transpose loads,
BatchNorm stats on device (combined across cores on host between two
launches), normalize+ReLU+transpose on device. All matmul operands bf16
(fp32 PSUM accumulation).
"""
import os, sys
sys.path.insert(0, '/opt/trn_rl_repo')
import numpy as np
import ml_dtypes
from contextlib import ExitStack

import concourse.bass as bass
import concourse.tile as tile
from concourse import bacc, mybir
from concourse.bass_utils import run_bass_kernel_spmd

B, V, E = 8, 12288, 98304
FIN, FOUT, K = 256, 256, 4
EPS = 1e-5
P = 128
GSZ = 64            # dst-group node window (S_w block width)
NVT = V // P        # 96 vtiles (group pairs)
GB = 4              # vtiles per gather batch
NB = NVT // GB      # 24 batches
SUP = 2048          # nodes per GEMM super-chunk
NSC = V // SUP      # 6 super-chunks
NCH = 24            # 512-node chunks for BN stats granularity

F32 = mybir.dt.float32
BF16 = mybir.dt.bfloat16
AF = mybir.ActivationFunctionType
BF = ml_dtypes.bfloat16

_cache = {}


def _build_schedule(edge_src, edge_dst, edge_weight):
    """Group edges by 64-node dst windows; pack full 128-lane gather slots per
    window, then merge the two windows' remainders of each vtile into one
    shared gather slot when they fit (saves gather rows).

    Returns:
      idx_np   [GT, P]   gather src row per (slot, lane)
      sw       [NMM, P, GSZ] scatter blocks (one per matmul)
      vt_mms   per vtile: list of (gather_slot, sw_slot, h)
      GT, NMM
    """
    g_of_e = edge_dst // GSZ
    order = np.argsort(g_of_e, kind='stable')
    NG = V // GSZ
    counts = np.bincount(g_of_e, minlength=NG)
    pos = np.zeros(NG + 1, np.int64)
    np.cumsum(counts, out=pos[1:])
    edges_of_g = [order[pos[g]:pos[g + 1]] for g in range(NG)]

    idx_rows = []
    sw_blocks = []
    vt_mms = [[] for _ in range(NVT)]

    def add_slot(parts):
        """parts: list of (edge_ids, h). One gather slot, one sw block per part."""
        gt = len(idx_rows)
        lanes = np.zeros(P, np.int32)
        off = 0
        entries = []
        for eids, h in parts:
            n = len(eids)
            lanes[off:off + n] = edge_src[eids]
            blk = np.zeros((P, GSZ), np.float32)
            blk[np.arange(off, off + n), edge_dst[eids] % GSZ] = edge_weight[eids]
            st = len(sw_blocks)
            sw_blocks.append(blk)
            entries.append((gt, st, h))
            off += n
        idx_rows.append(lanes)
        return entries

    for vt in range(NVT):
        g0, g1 = 2 * vt, 2 * vt + 1
        rema = []
        for g, h in ((g0, 0), (g1, 1)):
            eg = edges_of_g[g]
            nfull = len(eg) // P
            for s in range(nfull):
                vt_mms[vt] += add_slot([(eg[s * P:(s + 1) * P], h)])
            rem = eg[nfull * P:]
            if len(rem):
                rema.append((rem, h))
        if rema:
            if len(rema) == 2 and len(rema[0][0]) + len(rema[1][0]) <= P:
                vt_mms[vt] += add_slot(rema)
            else:
                for rem, h in rema:
                    vt_mms[vt] += add_slot([(rem, h)])

    idx_np = np.stack(idx_rows)                    # [GT, P]
    sw = np.stack(sw_blocks)                       # [NMM, P, GSZ]
    return idx_np, sw, vt_mms, len(idx_rows), len(sw_blocks)


def _fold_weights(weight):
    # out = sum_k T_k(L) x W_k ; T0=I, T1=L, T2=2L^2-1, T3=4L^3-3L
    # power basis z_j = L^j x :  out = sum_j z_j Wf_j
    W = weight.astype(np.float64)
    Wf = np.stack([W[0] - W[2], W[1] - 3.0 * W[3], 2.0 * W[2], 4.0 * W[3]])
    return Wf.reshape(K * FIN, FOUT).reshape(8, P, FOUT).astype(BF)


def _batches(vt_mms, GT):
    """Per gather batch of GB vtiles: (g0, ng, sw0, [(vt, mms_rebased)...]).
    Gather slots and sw slots are both contiguous in vtile order."""
    out = []
    for b in range(NB):
        vts = list(range(b * GB, (b + 1) * GB))
        g0 = vt_mms[vts[0]][0][0]
        sw0 = vt_mms[vts[0]][0][1]
        gmax, smax = 0, 0
        ventries = []
        for vt in vts:
            mms = [(gt - g0, st - sw0, h) for (gt, st, h) in vt_mms[vt]]
            gmax = max(gmax, max(m[0] for m in mms) + 1)
            smax = max(smax, max(m[1] for m in mms) + 1)
            ventries.append((vt, mms))
        out.append((g0, gmax, sw0, smax, ventries))
    return out


def _build_launch_a(vt_mms, GT, NMM):
    batches = _batches(vt_mms, GT)
    nc = bacc.Bacc("TRN2", target_bir_lowering=False, debug=False, num_devices=8)
    xb = nc.dram_tensor("xb", [V, FIN], BF16, kind="ExternalInput").ap()
    # int16 gather indices, wrapped: idx k of the global gather-lane order
    # (k = slot*128 + lane) lives at [k % 16, k // 16], replicated x8 rows.
    idx = nc.dram_tensor("idx", [P, GT * 8], mybir.dt.int16, kind="ExternalInput").ap()
    swt = nc.dram_tensor("swt", [P, NMM * GSZ], BF16, kind="ExternalInput").ap()
    wf = nc.dram_tensor("wf", [8, P, FOUT], BF16, kind="ExternalInput").ap()
    rawT = nc.dram_tensor("rawT", [2, P, V], BF16, kind="ExternalOutput").ap()
    stats = nc.dram_tensor("stats", [P, 4], F32, kind="ExternalOutput").ap()
    zd = [xb] + [nc.dram_tensor(f"z{j}", [V, FIN], BF16).ap() for j in (1, 2, 3)]

    with tile.TileContext(nc) as tc, ExitStack() as ctx:
        cpool = ctx.enter_context(tc.tile_pool(name="const", bufs=1))
        wf_t = cpool.tile([P, 8 * FOUT], BF16, tag="wf")
        nc.sync.dma_start(wf_t[:].rearrange("p (k o) -> p k o", k=8), wf.transpose([1, 0, 2]))

        # ---- Chebyshev (power-basis) recursion: z_j = L z_{j-1} ----
        with ExitStack() as rctx:
            rpool = rctx.enter_context(tc.tile_pool(name="rconst", bufs=1))
            idx_t = rpool.tile([P, GT * 8], mybir.dt.int16, tag="idx")
            nc.sync.dma_start(idx_t[:], idx[:, :])
            sw_t = rpool.tile([P, NMM * GSZ], BF16, tag="sw")
            # chunked resident load (pipelines with the first gathers)
            for (g0, ng, sw0, ns, _) in batches:
                nc.sync.dma_start(sw_t[:, sw0 * GSZ:(sw0 + ns) * GSZ],
                                  swt[:, sw0 * GSZ:(sw0 + ns) * GSZ])
            msgp = rctx.enter_context(tc.tile_pool(name="msgp", bufs=3))
            outp = rctx.enter_context(tc.tile_pool(name="outp", bufs=3))
            psp = rctx.enter_context(tc.tile_pool(name="psp", bufs=8, space="PSUM"))
            for j in (1, 2, 3):
                for (g0, ng, sw0, ns, ventries) in batches:
                    msg_t = msgp.tile([P, ng * FIN], BF16, tag="msg")
                    nc.gpsimd.dma_gather(
                        out_ap=msg_t[:].rearrange("p (t f) -> p t f", t=ng),
                        in_ap=zd[j - 1][:, :],
                        idxs_ap=idx_t[:, g0 * 8:(g0 + ng) * 8],
                        num_idxs=ng * P,
                        num_idxs_reg=ng * P,
                        elem_size=FIN,
                        single_packet=False)
                    zo = outp.tile([P, GB * FIN], BF16, tag="zo")
                    for i, (vt, mms) in enumerate(ventries):
                        ps = psp.tile([P, FIN], F32, tag="acc")
                        half_count = [sum(1 for m in mms if m[2] == hh) for hh in (0, 1)]
                        seen = [0, 0]
                        for (gt, st, h) in mms:
                            nc.tensor.matmul(
                                ps[h * GSZ:(h + 1) * GSZ, :],
                                sw_t[:, (sw0 + st) * GSZ:(sw0 + st + 1) * GSZ],
                                msg_t[:, gt * FIN:(gt + 1) * FIN],
                                start=(seen[h] == 0), stop=(seen[h] == half_count[h] - 1))
                            seen[h] += 1
                        if i % 2 == 0:
                            nc.scalar.activation(zo[:, i * FIN:(i + 1) * FIN], ps[:], AF.Copy)
                        else:
                            nc.vector.tensor_copy(zo[:, i * FIN:(i + 1) * FIN], ps[:])
                    b = ventries[0][0] // GB
                    nc.scalar.dma_start(
                        zd[j].rearrange("(b v p) f -> b p v f", v=GB, p=P)[b],
                        zo[:].rearrange("p (v f) -> p v f", v=GB))

        # ---- GEMM (zT via DMA-transpose) + BN stats ----
        with ExitStack() as gctx:
            ztp = gctx.enter_context(tc.tile_pool(name="ztp", bufs=3))
            rawp = gctx.enter_context(tc.tile_pool(name="rawp", bufs=3))
            psG = gctx.enter_context(tc.tile_pool(name="psG", bufs=4, space="PSUM"))
            sb = gctx.enter_context(tc.tile_pool(name="sb", bufs=1))
            stats_sb = sb.tile([P, 2 * NCH * 6], F32, tag="stats")
            for sc in range(NSC):
                zt = ztp.tile([P, 8 * SUP], BF16, tag="zt")
                for j in range(K):
                    for fh in range(2):
                        nc.sync.dma_start(
                            zt[:, (j * 2 + fh) * SUP:(j * 2 + fh + 1) * SUP],
                            zd[j][sc * SUP:(sc + 1) * SUP, fh * P:(fh + 1) * P],
                            transpose=True)
                raw_t = rawp.tile([P, 2 * SUP], BF16, tag="raw")
                for cc in range(SUP // 512):
                    for oh in range(2):
                        pg = psG.tile([P, 512], F32, tag="pg")
                        for kt in range(8):
                            nc.tensor.matmul(
                                pg[:], wf_t[:, kt * FOUT + oh * P: kt * FOUT + oh * P + P],
                                zt[:, kt * SUP + cc * 512: kt * SUP + (cc + 1) * 512],
                                start=(kt == 0), stop=(kt == 7))
                        c = sc * (SUP // 512) + cc
                        nc.vector.bn_stats(stats_sb[:, (oh * NCH + c) * 6:(oh * NCH + c) * 6 + 6], pg[:])
                        if (cc + oh) % 2 == 0:
                            nc.scalar.activation(raw_t[:, oh * SUP + cc * 512: oh * SUP + (cc + 1) * 512], pg[:], AF.Copy)
                        else:
                            nc.vector.tensor_copy(raw_t[:, oh * SUP + cc * 512: oh * SUP + (cc + 1) * 512], pg[:])
                for oh in range(2):
                    nc.scalar.dma_start(rawT[oh][:, sc * SUP:(sc + 1) * SUP],
                                        raw_t[:, oh * SUP:(oh + 1) * SUP])
            aggr = sb.tile([P, 4], F32, tag="aggr")
            for oh in range(2):
                nc.vector.bn_aggr(aggr[:, oh * 2:oh * 2 + 2],
                                  stats_sb[:, oh * NCH * 6:(oh + 1) * NCH * 6])
            # stats out: [mean_h0, ex2_h0, mean_h1, ex2_h1]
            so = sb.tile([P, 4], F32, tag="so")
            for oh in range(2):
                m = aggr[:, oh * 2:oh * 2 + 1]
                v_ = aggr[:, oh * 2 + 1:oh * 2 + 2]
                nc.vector.tensor_copy(so[:, oh * 2:oh * 2 + 1], m)
                nc.vector.tensor_tensor(out=so[:, oh * 2 + 1:oh * 2 + 2], in0=m, in1=m,
                                        op=mybir.AluOpType.mult)
                nc.vector.tensor_tensor(out=so[:, oh * 2 + 1:oh * 2 + 2],
                                        in0=so[:, oh * 2 + 1:oh * 2 + 2], in1=v_,
                                        op=mybir.AluOpType.add)
            nc.sync.dma_start(stats[:, :], so[:])
    nc.compile()
    return nc


def _build_launch_b():
    nc = bacc.Bacc("TRN2", target_bir_lowering=False, debug=False, num_devices=8)
    rawT = nc.dram_tensor("rawT", [2, P, V], BF16, kind="ExternalInput").ap()
    sc = nc.dram_tensor("sc", [P, 2], F32, kind="ExternalInput").ap()
    sh = nc.dram_tensor("sh", [P, 2], F32, kind="ExternalInput").ap()
    out = nc.dram_tensor("out", [V, FOUT], BF16, kind="ExternalOutput").ap()
    CH2 = 8           # vtiles per chunk
    NC2 = NVT // CH2  # 12 chunks
    from concourse.masks import make_identity
    with tile.TileContext(nc) as tc, ExitStack() as ctx:
        cpool = ctx.enter_context(tc.tile_pool(name="const", bufs=1))
        ident = cpool.tile([P, P], BF16, tag="id")
        make_identity(nc, ident[:])
        sc_t = cpool.tile([P, 2], F32, tag="sc")
        sh_t = cpool.tile([P, 2], F32, tag="sh")
        nc.sync.dma_start(sc_t[:], sc[:, :])
        nc.sync.dma_start(sh_t[:], sh[:, :])
        pool = ctx.enter_context(tc.tile_pool(name="sb", bufs=2))
        psp = ctx.enter_context(tc.tile_pool(name="ps", bufs=4, space="PSUM"))
        for c in range(NC2):
            nt = pool.tile([P, 2 * CH2 * P], BF16, tag="nt")
            for oh in range(2):
                nc.sync.dma_start(nt[:, oh * CH2 * P:(oh + 1) * CH2 * P],
                                  rawT[oh][:, c * CH2 * P:(c + 1) * CH2 * P])
            for oh in range(2):
                nc.scalar.activation(
                    nt[:, oh * CH2 * P:(oh + 1) * CH2 * P],
                    nt[:, oh * CH2 * P:(oh + 1) * CH2 * P],
                    AF.Relu, bias=sh_t[:, oh:oh + 1], scale=sc_t[:, oh:oh + 1])
            ot = pool.tile([P, CH2 * FOUT], BF16, tag="ot")
            for vt in range(CH2):
                pt = psp.tile([P, FOUT], BF16, tag="pt")
                for oh in range(2):
                    nc.tensor.transpose(
                        pt[:, oh * P:(oh + 1) * P],
                        nt[:, oh * CH2 * P + vt * P: oh * CH2 * P + (vt + 1) * P],
                        ident[:])
                if vt % 2 == 0:
                    nc.vector.tensor_copy(ot[:, vt * FOUT:(vt + 1) * FOUT], pt[:])
                else:
                    nc.scalar.activation(ot[:, vt * FOUT:(vt + 1) * FOUT], pt[:], AF.Copy)
            nc.scalar.dma_start(
                out.rearrange("(c vt p) f -> c p vt f", vt=CH2, p=P)[c],
                ot[:].rearrange("p (vt f) -> p vt f", vt=CH2))
    nc.compile()
    return nc


def kernel(x, edge_weight, weight, bias, gamma, beta, edge_src, edge_dst):
    x = np.asarray(x, np.float32)
    edge_weight = np.asarray(edge_weight, np.float32)
    weight = np.asarray(weight, np.float32)
    gamma = np.asarray(gamma, np.float32)
    beta = np.asarray(beta, np.float32)
    edge_src = np.asarray(edge_src, np.int32)
    edge_dst = np.asarray(edge_dst, np.int32)

    idx_np, sw, vt_mms, GT, NMM = _build_schedule(edge_src, edge_dst, edge_weight)
    key = ("A", GT, NMM, tuple(tuple(m) for mm in vt_mms for m in mm))
    if key not in _cache:
        _cache[key] = _build_launch_a(vt_mms, GT, NMM)
    ncA = _cache[key]
    if "B" not in _cache:
        _cache["B"] = _build_launch_b()
    ncB = _cache["B"]

    wf = _fold_weights(weight)
    swt = np.ascontiguousarray(sw.transpose(1, 0, 2)).reshape(P, NMM * GSZ).astype(BF)
    # wrapped int16 gather indices (see launch A): k = slot*128 + lane
    idx_flat = idx_np.reshape(GT * P).astype(np.int16)
    idx16 = np.ascontiguousarray(np.tile(idx_flat.reshape(-1, 16).T, (8, 1)))
    in_maps = []
    for b in range(B):
        in_maps.append({
            "xb": np.ascontiguousarray(x[b]).astype(BF),
            "idx": idx16, "swt": swt, "wf": wf,
        })
    resA = run_bass_kernel_spmd(ncA, in_maps, core_ids=list(range(B)))

    # host: combine BN stats across cores (equal counts -> simple average)
    st = np.stack([np.asarray(resA.results[b]["stats"]) for b in range(B)])  # [B, 128, 4]
    mean = st[:, :, [0, 2]].mean(0)                               # [128, 2]
    ex2 = st[:, :, [1, 3]].mean(0)
    var = ex2 - mean * mean
    g2 = gamma.reshape(2, P).T                                    # [128, 2]
    b2 = beta.reshape(2, P).T
    scale = (g2 / np.sqrt(var + EPS)).astype(np.float32)
    shift = (b2 - mean * scale).astype(np.float32)

    in_maps_b = [{"rawT": resA.results[b]["rawT"], "sc": scale, "sh": shift}
                 for b in range(B)]
    resB = run_bass_kernel_spmd(ncB, in_maps_b, core_ids=list(range(B)))
    out = np.stack([np.asarray(resB.results[b]["out"]).astype(np.float32) for b in range(B)])
    # bias cancels inside training-mode BN (shifts the mean only); gamma/beta applied above
    return out


# revision 12
# speedup vs baseline: 1.0073x; 1.0073x over previous
"""Trainium2 Bass kernel for nn_ConvBlock (Chebyshev graph conv + BatchNorm + ReLU).

Sharding: data-parallel over batch (B=8 -> 1 sample per NeuronCore).
Per core: power-basis Chebyshev recursion z_j = L z_{j-1} via batched
dma_gather row gathers (bf16) + PE scatter-matmuls (edge weights folded into
host-built one-hot scatter blocks, SBUF-resident), K-stacked GEMM with
host-folded power-basis weights consuming z^T via DMA-transpose loads,
BatchNorm stats on device (combined across cores on host between two
launches), normalize+ReLU+transpose on device. All matmul operands bf16
(fp32 PSUM accumulation).
"""
import os, sys
sys.path.insert(0, '/opt/trn_rl_repo')
import numpy as np
import ml_dtypes
from contextlib import ExitStack

import concourse.bass as bass
import concourse.tile as tile
from concourse import bacc, mybir
from concourse.bass_utils import run_bass_kernel_spmd

B, V, E = 8, 12288, 98304
FIN, FOUT, K = 256, 256, 4
EPS = 1e-5
P = 128
GSZ = 64            # dst-group node window (S_w block width)
NVT = V // P        # 96 vtiles (group pairs)
GB = 4              # vtiles per gather batch
NB = NVT // GB      # 24 batches
SUP = 2048          # nodes per GEMM super-chunk
NSC = V // SUP      # 6 super-chunks
NCH = 24            # 512-node chunks for BN stats granularity

F32 = mybir.dt.float32
BF16 = mybir.dt.bfloat16
AF = mybir.ActivationFunctionType
BF = ml_dtypes.bfloat16

_cache = {}


def _build_schedule(edge_src, edge_dst, edge_weight):
    """Group edges by 64-node dst windows; pack full 128-lane gather slots per
    window, then merge the two windows' remainders of each vtile into one
    shared gather slot when they fit (saves gather rows).

    Returns:
      idx_np   [GT, P]   gather src row per (slot, lane)
      sw       [NMM, P, GSZ] scatter blocks (one per matmul)
      vt_mms   per vtile: list of (gather_slot, sw_slot, h)
      GT, NMM
    """
    g_of_e = edge_dst // GSZ
    order = np.argsort(g_of_e, kind='stable')
    NG = V // GSZ
    counts = np.bincount(g_of_e, minlength=NG)
    pos = np.zeros(NG + 1, np.int64)
    np.cumsum(counts, out=pos[1:])
    edges_of_g = [order[pos[g]:pos[g + 1]] for g in range(NG)]

    idx_rows = []
    sw_blocks = []
    vt_mms = [[] for _ in range(NVT)]

    def add_slot(parts):
        """parts: list of (edge_ids, h). One gather slot, one sw block per part."""
        gt = len(idx_rows)
        lanes = np.zeros(P, np.int32)
        off = 0
        entries = []
        for eids, h in parts:
            n = len(eids)
            lanes[off:off + n] = edge_src[eids]
            blk = np.zeros((P, GSZ), np.float32)
            blk[np.arange(off, off + n), edge_dst[eids] % GSZ] = edge_weight[eids]
            st = len(sw_blocks)
            sw_blocks.append(blk)
            entries.append((gt, st, h))
            off += n
        idx_rows.append(lanes)
        return entries

    for vt in range(NVT):
        g0, g1 = 2 * vt, 2 * vt + 1
        rema = []
        for g, h in ((g0, 0), (g1, 1)):
            eg = edges_of_g[g]
            nfull = len(eg) // P
            for s in range(nfull):
                vt_mms[vt] += add_slot([(eg[s * P:(s + 1) * P], h)])
            rem = eg[nfull * P:]
            if len(rem):
                rema.append((rem, h))
        if rema:
            if len(rema) == 2 and len(rema[0][0]) + len(rema[1][0]) <= P:
                vt_mms[vt] += add_slot(rema)
            else:
                for rem, h in rema:
                    vt_mms[vt] += add_slot([(rem, h)])

    idx_np = np.stack(idx_rows)                    # [GT, P]
    sw = np.stack(sw_blocks)                       # [NMM, P, GSZ]
    return idx_np, sw, vt_mms, len(idx_rows), len(sw_blocks)


def _fold_weights(weight):
    # out = sum_k T_k(L) x W_k ; T0=I, T1=L, T2=2L^2-1, T3=4L^3-3L
    # power basis z_j = L^j x :  out = sum_j z_j Wf_j
    W = weight.astype(np.float64)
    Wf = np.stack([W[0] - W[2], W[1] - 3.0 * W[3], 2.0 * W[2], 4.0 * W[3]])
    return Wf.reshape(K * FIN, FOUT).reshape(8, P, FOUT).astype(BF)


def _batches(vt_mms, GT):
    """Per gather batch of GB vtiles: (g0, ng, sw0, [(vt, mms_rebased)...]).
    Gather slots and sw slots are both contiguous in vtile order."""
    out = []
    for b in range(NB):
        vts = list(range(b * GB, (b + 1) * GB))
        g0 = vt_mms[vts[0]][0][0]
        sw0 = vt_mms[vts[0]][0][1]
        gmax, smax = 0, 0
        ventries = []
        for vt in vts:
            mms = [(gt - g0, st - sw0, h) for (gt, st, h) in vt_mms[vt]]
            gmax = max(gmax, max(m[0] for m in mms) + 1)
            smax = max(smax, max(m[1] for m in mms) + 1)
            ventries.append((vt, mms))
        out.append((g0, gmax, sw0, smax, ventries))
    return out


def _build_launch_a(vt_mms, GT, NMM):
    batches = _batches(vt_mms, GT)
    nc = bacc.Bacc("TRN2", target_bir_lowering=False, debug=False, num_devices=8)
    xb = nc.dram_tensor("xb", [V, FIN], BF16, kind="ExternalInput").ap()
    # int16 gather indices, wrapped: idx k of the global gather-lane order
    # (k = slot*128 + lane) lives at [k % 16, k // 16], replicated x8 rows.
    idx = nc.dram_tensor("idx", [P, GT * 8], mybir.dt.int16, kind="ExternalInput").ap()
    swt = nc.dram_tensor("swt", [P, NMM * GSZ], BF16, kind="ExternalInput").ap()
    wf = nc.dram_tensor("wf", [8, P, FOUT], BF16, kind="ExternalInput").ap()
    rawT = nc.dram_tensor("rawT", [2, P, V], BF16, kind="ExternalOutput").ap()
    stats = nc.dram_tensor("stats", [P, 4], F32, kind="ExternalOutput").ap()
    zd = [xb] + [nc.dram_tensor(f"z{j}", [V, FIN], BF16).ap() for j in (1, 2, 3)]

    with tile.TileContext(nc) as tc, ExitStack() as ctx:
        cpool = ctx.enter_context(tc.tile_pool(name="const", bufs=1))
        wf_t = cpool.tile([P, 8 * FOUT], BF16, tag="wf")
        nc.sync.dma_start(wf_t[:].rearrange("p (k o) -> p k o", k=8), wf.transpose([1, 0, 2]))

        # ---- Chebyshev (power-basis) recursion: z_j = L z_{j-1} ----
        with ExitStack() as rctx:
            rpool = rctx.enter_context(tc.tile_pool(name="rconst", bufs=1))
            idx_t = rpool.tile([P, GT * 8], mybir.dt.int16, tag="idx")
            nc.sync.dma_start(idx_t[:], idx[:, :])
            sw_t = rpool.tile([P, NMM * GSZ], BF16, tag="sw")
            # chunked resident load (pipelines with the first gathers)
            for (g0, ng, sw0, ns, _) in batches:
                nc.sync.dma_start(sw_t[:, sw0 * GSZ:(sw0 + ns) * GSZ],
                                  swt[:, sw0 * GSZ:(sw0 + ns) * GSZ])
            msgp = rctx.enter_context(tc.tile_pool(name="msgp", bufs=3))
            outp = rctx.enter_context(tc.tile_pool(name="outp", bufs=3))
            psp = rctx.enter_context(tc.tile_pool(name="psp", bufs=8, space="PSUM"))
            for j in (1, 2, 3):
                for (g0, ng, sw0, ns, ventries) in batches:
                    msg_t = msgp.tile([P, ng * FIN], BF16, tag="msg")
                    nc.gpsimd.dma_gather(
                        out_ap=msg_t[:].rearrange("p (t f) -> p t f", t=ng),
                        in_ap=zd[j - 1][:, :],
                        idxs_ap=idx_t[:, g0 * 8:(g0 + ng) * 8],
                        num_idxs=ng * P,
                        num_idxs_reg=ng * P,
                        elem_size=FIN,
                        single_packet=False)
                    zo = outp.tile([P, GB * FIN], BF16, tag="zo")
                    for i, (vt, mms) in enumerate(ventries):
                        ps = psp.tile([P, FIN], F32, tag="acc")
                        half_count = [sum(1 for m in mms if m[2] == hh) for hh in (0, 1)]
                        seen = [0, 0]
                        for (gt, st, h) in mms:
                            nc.tensor.matmul(
                                ps[h * GSZ:(h + 1) * GSZ, :],
                                sw_t[:, (sw0 + st) * GSZ:(sw0 + st + 1) * GSZ],
                                msg_t[:, gt * FIN:(gt + 1) * FIN],
                                start=(seen[h] == 0), stop=(seen[h] == half_count[h] - 1))
                            seen[h] += 1
                        if i % 2 == 0:
                            nc.scalar.activation(zo[:, i * FIN:(i + 1) * FIN], ps[:], AF.Copy)
                        else:
                            nc.vector.tensor_copy(zo[:, i * FIN:(i + 1) * FIN], ps[:])
                    b = ventries[0][0] // GB
                    nc.scalar.dma_start(
                        zd[j].rearrange("(b v p) f -> b p v f", v=GB, p=P)[b],
                        zo[:].rearrange("p (v f) -> p v f", v=GB))

        # ---- GEMM (zT via DMA-transpose, kt-outer so MMs trail each dmaT) ----
        with ExitStack() as gctx:
            ztp = gctx.enter_context(tc.tile_pool(name="ztp", bufs=2))
            rawp = gctx.enter_context(tc.tile_pool(name="rawp", bufs=2))
            psG = gctx.enter_context(tc.tile_pool(name="psG", bufs=8, space="PSUM"))
            sb = gctx.enter_context(tc.tile_pool(name="sb", bufs=1))
            stats_sb = sb.tile([P, 2 * NCH * 6], F32, tag="stats")
            for sc in range(NSC):
                zt = ztp.tile([P, 8 * SUP], BF16, tag="zt")
                pgs = [psG.tile([P, 512], F32, tag="pg", name=f"pg{sc}_{i}")
                       for i in range(8)]
                for kt in range(8):           # kt = (j, fh)
                    j, fh = kt // 2, kt % 2
                    nc.sync.dma_start(
                        zt[:, kt * SUP:(kt + 1) * SUP],
                        zd[j][sc * SUP:(sc + 1) * SUP, fh * P:(fh + 1) * P],
                        transpose=True)
                    for cc in range(SUP // 512):
                        for oh in range(2):
                            nc.tensor.matmul(
                                pgs[cc * 2 + oh][:],
                                wf_t[:, kt * FOUT + oh * P: kt * FOUT + oh * P + P],
                                zt[:, kt * SUP + cc * 512: kt * SUP + (cc + 1) * 512],
                                start=(kt == 0), stop=(kt == 7))
                raw_t = rawp.tile([P, 2 * SUP], BF16, tag="raw")
                for cc in range(SUP // 512):
                    for oh in range(2):
                        pg = pgs[cc * 2 + oh]
                        c = sc * (SUP // 512) + cc
                        nc.vector.bn_stats(stats_sb[:, (oh * NCH + c) * 6:(oh * NCH + c) * 6 + 6], pg[:])
                        if (cc + oh) % 2 == 0:
                            nc.scalar.activation(raw_t[:, oh * SUP + cc * 512: oh * SUP + (cc + 1) * 512], pg[:], AF.Copy)
                        else:
                            nc.vector.tensor_copy(raw_t[:, oh * SUP + cc * 512: oh * SUP + (cc + 1) * 512], pg[:])
                for oh in range(2):
                    nc.scalar.dma_start(rawT[oh][:, sc * SUP:(sc + 1) * SUP],
                                        raw_t[:, oh * SUP:(oh + 1) * SUP])
            aggr = sb.tile([P, 4], F32, tag="aggr")
            for oh in range(2):
                nc.vector.bn_aggr(aggr[:, oh * 2:oh * 2 + 2],
                                  stats_sb[:, oh * NCH * 6:(oh + 1) * NCH * 6])
            # stats out: [mean_h0, ex2_h0, mean_h1, ex2_h1]
            so = sb.tile([P, 4], F32, tag="so")
            for oh in range(2):
                m = aggr[:, oh * 2:oh * 2 + 1]
                v_ = aggr[:, oh * 2 + 1:oh * 2 + 2]
                nc.vector.tensor_copy(so[:, oh * 2:oh * 2 + 1], m)
                nc.vector.tensor_tensor(out=so[:, oh * 2 + 1:oh * 2 + 2], in0=m, in1=m,
                                        op=mybir.AluOpType.mult)
                nc.vector.tensor_tensor(out=so[:, oh * 2 + 1:oh * 2 + 2],
                                        in0=so[:, oh * 2 + 1:oh * 2 + 2], in1=v_,
                                        op=mybir.AluOpType.add)
            nc.sync.dma_start(stats[:, :], so[:])
    nc.compile()
    return nc


def _build_launch_b():
    nc = bacc.Bacc("TRN2", target_bir_lowering=False, debug=False, num_devices=8)
    rawT = nc.dram_tensor("rawT", [2, P, V], BF16, kind="ExternalInput").ap()
    sc = nc.dram_tensor("sc", [P, 2], F32, kind="ExternalInput").ap()
    sh = nc.dram_tensor("sh", [P, 2], F32, kind="ExternalInput").ap()
    out = nc.dram_tensor("out", [V, FOUT], BF16, kind="ExternalOutput").ap()
    CH2 = 8           # vtiles per chunk
    NC2 = NVT // CH2  # 12 chunks
    from concourse.masks import make_identity
    with tile.TileContext(nc) as tc, ExitStack() as ctx:
        cpool = ctx.enter_context(tc.tile_pool(name="const", bufs=1))
        ident = cpool.tile([P, P], BF16, tag="id")
        make_identity(nc, ident[:])
        sc_t = cpool.tile([P, 2], F32, tag="sc")
        sh_t = cpool.tile([P, 2], F32, tag="sh")
        nc.sync.dma_start(sc_t[:], sc[:, :])
        nc.sync.dma_start(sh_t[:], sh[:, :])
        pool = ctx.enter_context(tc.tile_pool(name="sb", bufs=2))
        psp = ctx.enter_context(tc.tile_pool(name="ps", bufs=4, space="PSUM"))
        for c in range(NC2):
            nt = pool.tile([P, 2 * CH2 * P], BF16, tag="nt")
            for oh in range(2):
                nc.sync.dma_start(nt[:, oh * CH2 * P:(oh + 1) * CH2 * P],
                                  rawT[oh][:, c * CH2 * P:(c + 1) * CH2 * P])
            for oh in range(2):
                nc.scalar.activation(
                    nt[:, oh * CH2 * P:(oh + 1) * CH2 * P],
                    nt[:, oh * CH2 * P:(oh + 1) * CH2 * P],
                    AF.Relu, bias=sh_t[:, oh:oh + 1], scale=sc_t[:, oh:oh + 1])
            ot = pool.tile([P, CH2 * FOUT], BF16, tag="ot")
            for vt in range(CH2):
                pt = psp.tile([P, FOUT], BF16, tag="pt")
                for oh in range(2):
                    nc.tensor.transpose(
                        pt[:, oh * P:(oh + 1) * P],
                        nt[:, oh * CH2 * P + vt * P: oh * CH2 * P + (vt + 1) * P],
                        ident[:])
                if vt % 2 == 0:
                    nc.vector.tensor_copy(ot[:, vt * FOUT:(vt + 1) * FOUT], pt[:])
                else:
                    nc.scalar.activation(ot[:, vt * FOUT:(vt + 1) * FOUT], pt[:], AF.Copy)
            nc.scalar.dma_start(
                out.rearrange("(c vt p) f -> c p vt f", vt=CH2, p=P)[c],
                ot[:].rearrange("p (vt f) -> p vt f", vt=CH2))
    nc.compile()
    return nc


def kernel(x, edge_weight, weight, bias, gamma, beta, edge_src, edge_dst):
    x = np.asarray(x, np.float32)
    edge_weight = np.asarray(edge_weight, np.float32)
    weight = np.asarray(weight, np.float32)
    gamma = np.asarray(gamma, np.float32)
    beta = np.asarray(beta, np.float32)
    edge_src = np.asarray(edge_src, np.int32)
    edge_dst = np.asarray(edge_dst, np.int32)

    idx_np, sw, vt_mms, GT, NMM = _build_schedule(edge_src, edge_dst, edge_weight)
    key = ("A", GT, NMM, tuple(tuple(m) for mm in vt_mms for m in mm))
    if key not in _cache:
        _cache[key] = _build_launch_a(vt_mms, GT, NMM)
    ncA = _cache[key]
    if "B" not in _cache:
        _cache["B"] = _build_launch_b()
    ncB = _cache["B"]

    wf = _fold_weights(weight)
    swt = np.ascontiguousarray(sw.transpose(1, 0, 2)).reshape(P, NMM * GSZ).astype(BF)
    # wrapped int16 gather indices (see launch A): k = slot*128 + lane
    idx_flat = idx_np.reshape(GT * P).astype(np.int16)
    idx16 = np.ascontiguousarray(np.tile(idx_flat.reshape(-1, 16).T, (8, 1)))
    in_maps = []
    for b in range(B):
        in_maps.append({
            "xb": np.ascontiguousarray(x[b]).astype(BF),
            "idx": idx16, "swt": swt, "wf": wf,
        })
    resA = run_bass_kernel_spmd(ncA, in_maps, core_ids=list(range(B)))

    # host: combine BN stats across cores (equal counts -> simple average)
    st = np.stack([np.asarray(resA.results[b]["stats"]) for b in range(B)])  # [B, 128, 4]
    mean = st[:, :, [0, 2]].mean(0)                               # [128, 2]
    ex2 = st[:, :, [1, 3]].mean(0)
    var = ex2 - mean * mean
    g2 = gamma.reshape(2, P).T                                    # [128, 2]
    b2 = beta.reshape(2, P).T
    scale = (g2 / np.sqrt(var + EPS)).astype(np.float32)
    shift = (b2 - mean * scale).astype(np.float32)

    in_maps_b = [{"rawT": resA.results[b]["rawT"], "sc": scale, "sh": shift}
                 for b in range(B)]
    resB = run_bass_kernel_spmd(ncB, in_maps_b, core_ids=list(range(B)))
    out = np.stack([np.asarray(resB.results[b]["out"]).astype(np.float32) for b in range(B)])
    # bias cancels inside training-mode BN (shifts the mean only); gamma/beta applied above
    return out


# revision 13
# speedup vs baseline: 1.0073x; 1.0001x over previous
"""Trainium2 Bass kernel for nn_ConvBlock (Chebyshev graph conv + BatchNorm + ReLU).

Sharding: data-parallel over batch (B=8 -> 1 sample per NeuronCore).
Per core: power-basis Chebyshev recursion z_j = L z_{j-1} via batched
dma_gather row gathers (bf16) + PE scatter-matmuls (edge weights folded into
host-built one-hot scatter blocks, SBUF-resident), K-stacked GEMM with
host-folded power-basis weights consuming z^T via DMA-transpose loads,
BatchNorm stats on device (combined across cores on host between two
launches), normalize+ReLU+transpose on device. All matmul operands bf16
(fp32 PSUM accumulation).
"""
import os, sys
sys.path.insert(0, '/opt/trn_rl_repo')
import numpy as np
import ml_dtypes
from contextlib import ExitStack

import concourse.bass as bass
import concourse.tile as tile
from concourse import bacc, mybir
from concourse.bass_utils import run_bass_kernel_spmd

B, V, E = 8, 12288, 98304
FIN, FOUT, K = 256, 256, 4
EPS = 1e-5
P = 128
GSZ = 64            # dst-group node window (S_w block width)
NVT = V // P        # 96 vtiles (group pairs)
GB = 4              # vtiles per gather batch
NB = NVT // GB      # 24 batches
SUP = 2048          # nodes per GEMM super-chunk
NSC = V // SUP      # 6 super-chunks
NCH = 24            # 512-node chunks for BN stats granularity

F32 = mybir.dt.float32
BF16 = mybir.dt.bfloat16
AF = mybir.ActivationFunctionType
BF = ml_dtypes.bfloat16

_cache = {}


def _build_schedule(edge_src, edge_dst, edge_weight):
    """Group edges by 64-node dst windows; pack full 128-lane gather slots per
    window, then merge the two windows' remainders of each vtile into one
    shared gather slot when they fit (saves gather rows).

    Returns:
      idx_np   [GT, P]   gather src row per (slot, lane)
      sw       [NMM, P, GSZ] scatter blocks (one per matmul)
      vt_mms   per vtile: list of (gather_slot, sw_slot, h)
      GT, NMM
    """
    g_of_e = edge_dst // GSZ
    order = np.argsort(g_of_e, kind='stable')
    NG = V // GSZ
    counts = np.bincount(g_of_e, minlength=NG)
    pos = np.zeros(NG + 1, np.int64)
    np.cumsum(counts, out=pos[1:])
    edges_of_g = [order[pos[g]:pos[g + 1]] for g in range(NG)]

    idx_rows = []
    sw_blocks = []
    vt_mms = [[] for _ in range(NVT)]

    def add_slot(parts):
        """parts: list of (edge_ids, h). One gather slot, one sw block per part."""
        gt = len(idx_rows)
        lanes = np.zeros(P, np.int32)
        off = 0
        entries = []
        for eids, h in parts:
            n = len(eids)
            lanes[off:off + n] = edge_src[eids]
            blk = np.zeros((P, GSZ), np.float32)
            blk[np.arange(off, off + n), edge_dst[eids] % GSZ] = edge_weight[eids]
            st = len(sw_blocks)
            sw_blocks.append(blk)
            entries.append((gt, st, h))
            off += n
        idx_rows.append(lanes)
        return entries

    for vt in range(NVT):
        g0, g1 = 2 * vt, 2 * vt + 1
        rema = []
        for g, h in ((g0, 0), (g1, 1)):
            eg = edges_of_g[g]
            nfull = len(eg) // P
            for s in range(nfull):
                vt_mms[vt] += add_slot([(eg[s * P:(s + 1) * P], h)])
            rem = eg[nfull * P:]
            if len(rem):
                rema.append((rem, h))
        if rema:
            if len(rema) == 2 and len(rema[0][0]) + len(rema[1][0]) <= P:
                vt_mms[vt] += add_slot(rema)
            else:
                for rem, h in rema:
                    vt_mms[vt] += add_slot([(rem, h)])

    idx_np = np.stack(idx_rows)                    # [GT, P]
    sw = np.stack(sw_blocks)                       # [NMM, P, GSZ]
    return idx_np, sw, vt_mms, len(idx_rows), len(sw_blocks)


def _fold_weights(weight):
    # out = sum_k T_k(L) x W_k ; T0=I, T1=L, T2=2L^2-1, T3=4L^3-3L
    # power basis z_j = L^j x :  out = sum_j z_j Wf_j
    W = weight.astype(np.float64)
    Wf = np.stack([W[0] - W[2], W[1] - 3.0 * W[3], 2.0 * W[2], 4.0 * W[3]])
    return Wf.reshape(K * FIN, FOUT).reshape(8, P, FOUT).astype(BF)


def _batches(vt_mms, GT):
    """Per gather batch of GB vtiles: (g0, ng, sw0, [(vt, mms_rebased)...]).
    Gather slots and sw slots are both contiguous in vtile order."""
    out = []
    for b in range(NB):
        vts = list(range(b * GB, (b + 1) * GB))
        g0 = vt_mms[vts[0]][0][0]
        sw0 = vt_mms[vts[0]][0][1]
        gmax, smax = 0, 0
        ventries = []
        for vt in vts:
            mms = [(gt - g0, st - sw0, h) for (gt, st, h) in vt_mms[vt]]
            gmax = max(gmax, max(m[0] for m in mms) + 1)
            smax = max(smax, max(m[1] for m in mms) + 1)
            ventries.append((vt, mms))
        out.append((g0, gmax, sw0, smax, ventries))
    return out


def _build_launch_a(vt_mms, GT, NMM):
    batches = _batches(vt_mms, GT)
    nc = bacc.Bacc("TRN2", target_bir_lowering=False, debug=False, num_devices=8)
    xb = nc.dram_tensor("xb", [V, FIN], BF16, kind="ExternalInput").ap()
    # int16 gather indices, wrapped: idx k of the global gather-lane order
    # (k = slot*128 + lane) lives at [k % 16, k // 16], replicated x8 rows.
    idx = nc.dram_tensor("idx", [P, GT * 8], mybir.dt.int16, kind="ExternalInput").ap()
    swt = nc.dram_tensor("swt", [P, NMM * GSZ], BF16, kind="ExternalInput").ap()
    wf = nc.dram_tensor("wf", [8, P, FOUT], BF16, kind="ExternalInput").ap()
    rawT = nc.dram_tensor("rawT", [2, P, V], BF16, kind="ExternalOutput").ap()
    stats = nc.dram_tensor("stats", [P, 4], F32, kind="ExternalOutput").ap()
    zd = [xb] + [nc.dram_tensor(f"z{j}", [V, FIN], BF16).ap() for j in (1, 2, 3)]

    with tile.TileContext(nc) as tc, ExitStack() as ctx:
        cpool = ctx.enter_context(tc.tile_pool(name="const", bufs=1))
        wf_t = cpool.tile([P, 8 * FOUT], BF16, tag="wf")
        nc.sync.dma_start(wf_t[:].rearrange("p (k o) -> p k o", k=8), wf.transpose([1, 0, 2]))

        # ---- Chebyshev (power-basis) recursion: z_j = L z_{j-1} ----
        with ExitStack() as rctx:
            rpool = rctx.enter_context(tc.tile_pool(name="rconst", bufs=1))
            idx_t = rpool.tile([P, GT * 8], mybir.dt.int16, tag="idx")
            nc.sync.dma_start(idx_t[:], idx[:, :])
            sw_t = rpool.tile([P, NMM * GSZ], BF16, tag="sw")
            # chunked resident load (pipelines with the first gathers)
            for (g0, ng, sw0, ns, _) in batches:
                nc.sync.dma_start(sw_t[:, sw0 * GSZ:(sw0 + ns) * GSZ],
                                  swt[:, sw0 * GSZ:(sw0 + ns) * GSZ])
            msgp = rctx.enter_context(tc.tile_pool(name="msgp", bufs=3))
            outp = rctx.enter_context(tc.tile_pool(name="outp", bufs=3))
            psp = rctx.enter_context(tc.tile_pool(name="psp", bufs=8, space="PSUM"))
            for j in (1, 2, 3):
                for (g0, ng, sw0, ns, ventries) in batches:
                    msg_t = msgp.tile([P, ng * FIN], BF16, tag="msg")
                    nc.gpsimd.dma_gather(
                        out_ap=msg_t[:].rearrange("p (t f) -> p t f", t=ng),
                        in_ap=zd[j - 1][:, :],
                        idxs_ap=idx_t[:, g0 * 8:(g0 + ng) * 8],
                        num_idxs=ng * P,
                        num_idxs_reg=ng * P,
                        elem_size=FIN,
                        single_packet=False)
                    zo = outp.tile([P, GB * FIN], BF16, tag="zo")
                    for i, (vt, mms) in enumerate(ventries):
                        ps = psp.tile([P, FIN], F32, tag="acc")
                        half_count = [sum(1 for m in mms if m[2] == hh) for hh in (0, 1)]
                        seen = [0, 0]
                        for (gt, st, h) in mms:
                            nc.tensor.matmul(
                                ps[h * GSZ:(h + 1) * GSZ, :],
                                sw_t[:, (sw0 + st) * GSZ:(sw0 + st + 1) * GSZ],
                                msg_t[:, gt * FIN:(gt + 1) * FIN],
                                start=(seen[h] == 0), stop=(seen[h] == half_count[h] - 1))
                            seen[h] += 1
                        if i % 2 == 0:
                            nc.scalar.activation(zo[:, i * FIN:(i + 1) * FIN], ps[:], AF.Copy)
                        else:
                            nc.vector.tensor_copy(zo[:, i * FIN:(i + 1) * FIN], ps[:])
                    b = ventries[0][0] // GB
                    nc.scalar.dma_start(
                        zd[j].rearrange("(b v p) f -> b p v f", v=GB, p=P)[b],
                        zo[:].rearrange("p (v f) -> p v f", v=GB))

        # ---- GEMM (zT via DMA-transpose, kt-outer so MMs trail each dmaT) ----
        with ExitStack() as gctx:
            ztp = gctx.enter_context(tc.tile_pool(name="ztp", bufs=3))
            rawp = gctx.enter_context(tc.tile_pool(name="rawp", bufs=2))
            psG = gctx.enter_context(tc.tile_pool(name="psG", bufs=8, space="PSUM"))
            sb = gctx.enter_context(tc.tile_pool(name="sb", bufs=1))
            stats_sb = sb.tile([P, 2 * NCH * 6], F32, tag="stats")
            for sc in range(NSC):
                zt = ztp.tile([P, 8 * SUP], BF16, tag="zt")
                pgs = [psG.tile([P, 512], F32, tag="pg", name=f"pg{sc}_{i}")
                       for i in range(8)]
                for kt in range(8):           # kt = (j, fh)
                    j, fh = kt // 2, kt % 2
                    nc.sync.dma_start(
                        zt[:, kt * SUP:(kt + 1) * SUP],
                        zd[j][sc * SUP:(sc + 1) * SUP, fh * P:(fh + 1) * P],
                        transpose=True)
                    for cc in range(SUP // 512):
                        for oh in range(2):
                            nc.tensor.matmul(
                                pgs[cc * 2 + oh][:],
                                wf_t[:, kt * FOUT + oh * P: kt * FOUT + oh * P + P],
                                zt[:, kt * SUP + cc * 512: kt * SUP + (cc + 1) * 512],
                                start=(kt == 0), stop=(kt == 7))
                raw_t = rawp.tile([P, 2 * SUP], BF16, tag="raw")
                for cc in range(SUP // 512):
                    for oh in range(2):
                        pg = pgs[cc * 2 + oh]
                        c = sc * (SUP // 512) + cc
                        nc.vector.bn_stats(stats_sb[:, (oh * NCH + c) * 6:(oh * NCH + c) * 6 + 6], pg[:])
                        if (cc + oh) % 2 == 0:
                            nc.scalar.activation(raw_t[:, oh * SUP + cc * 512: oh * SUP + (cc + 1) * 512], pg[:], AF.Copy)
                        else:
                            nc.vector.tensor_copy(raw_t[:, oh * SUP + cc * 512: oh * SUP + (cc + 1) * 512], pg[:])
                for oh in range(2):
                    nc.scalar.dma_start(rawT[oh][:, sc * SUP:(sc + 1) * SUP],
                                        raw_t[:, oh * SUP:(oh + 1) * SUP])
            aggr = sb.tile([P, 4], F32, tag="aggr")
            for oh in range(2):
                nc.vector.bn_aggr(aggr[:, oh * 2:oh * 2 + 2],
                                  stats_sb[:, oh * NCH * 6:(oh + 1) * NCH * 6])
            # stats out: [mean_h0, ex2_h0, mean_h1, ex2_h1]
            so = sb.tile([P, 4], F32, tag="so")
            for oh in range(2):
                m = aggr[:, oh * 2:oh * 2 + 1]
                v_ = aggr[:, oh * 2 + 1:oh * 2 + 2]
                nc.vector.tensor_copy(so[:, oh * 2:oh * 2 + 1], m)
                nc.vector.tensor_tensor(out=so[:, oh * 2 + 1:oh * 2 + 2], in0=m, in1=m,
                                        op=mybir.AluOpType.mult)
                nc.vector.tensor_tensor(out=so[:, oh * 2 + 1:oh * 2 + 2],
                                        in0=so[:, oh * 2 + 1:oh * 2 + 2], in1=v_,
                                        op=mybir.AluOpType.add)
            nc.sync.dma_start(stats[:, :], so[:])
    nc.compile()
    return nc


def _build_launch_b():
    nc = bacc.Bacc("TRN2", target_bir_lowering=False, debug=False, num_devices=8)
    rawT = nc.dram_tensor("rawT", [2, P, V], BF16, kind="ExternalInput").ap()
    sc = nc.dram_tensor("sc", [P, 2], F32, kind="ExternalInput").ap()
    sh = nc.dram_tensor("sh", [P, 2], F32, kind="ExternalInput").ap()
    out = nc.dram_tensor("out", [V, FOUT], BF16, kind="ExternalOutput").ap()
    CH2 = 8           # vtiles per chunk
    NC2 = NVT // CH2  # 12 chunks
    from concourse.masks import make_identity
    with tile.TileContext(nc) as tc, ExitStack() as ctx:
        cpool = ctx.enter_context(tc.tile_pool(name="const", bufs=1))
        ident = cpool.tile([P, P], BF16, tag="id")
        make_identity(nc, ident[:])
        sc_t = cpool.tile([P, 2], F32, tag="sc")
        sh_t = cpool.tile([P, 2], F32, tag="sh")
        nc.sync.dma_start(sc_t[:], sc[:, :])
        nc.sync.dma_start(sh_t[:], sh[:, :])
        pool = ctx.enter_context(tc.tile_pool(name="sb", bufs=2))
        psp = ctx.enter_context(tc.tile_pool(name="ps", bufs=4, space="PSUM"))
        for c in range(NC2):
            nt = pool.tile([P, 2 * CH2 * P], BF16, tag="nt")
            for oh in range(2):
                nc.sync.dma_start(nt[:, oh * CH2 * P:(oh + 1) * CH2 * P],
                                  rawT[oh][:, c * CH2 * P:(c + 1) * CH2 * P])
            for oh in range(2):
                nc.scalar.activation(
                    nt[:, oh * CH2 * P:(oh + 1) * CH2 * P],
                    nt[:, oh * CH2 * P:(oh + 1) * CH2 * P],
                    AF.Relu, bias=sh_t[:, oh:oh + 1], scale=sc_t[:, oh:oh + 1])
            ot = pool.tile([P, CH2 * FOUT], BF16, tag="ot")
            for vt in range(CH2):
                pt = psp.tile([P, FOUT], BF16, tag="pt")
                for oh in range(2):
                    nc.tensor.transpose(
                        pt[:, oh * P:(oh + 1) * P],
                        nt[:, oh * CH2 * P + vt * P: oh * CH2 * P + (vt + 1) * P],
                        ident[:])
                if vt % 2 == 0:
                    nc.vector.tensor_copy(ot[:, vt * FOUT:(vt + 1) * FOUT], pt[:])
                else:
                    nc.scalar.activation(ot[:, vt * FOUT:(vt + 1) * FOUT], pt[:], AF.Copy)
            nc.scalar.dma_start(
                out.rearrange("(c vt p) f -> c p vt f", vt=CH2, p=P)[c],
                ot[:].rearrange("p (vt f) -> p vt f", vt=CH2))
    nc.compile()
    return nc


def kernel(x, edge_weight, weight, bias, gamma, beta, edge_src, edge_dst):
    x = np.asarray(x, np.float32)
    edge_weight = np.asarray(edge_weight, np.float32)
    weight = np.asarray(weight, np.float32)
    gamma = np.asarray(gamma, np.float32)
    beta = np.asarray(beta, np.float32)
    edge_src = np.asarray(edge_src, np.int32)
    edge_dst = np.asarray(edge_dst, np.int32)

    idx_np, sw, vt_mms, GT, NMM = _build_schedule(edge_src, edge_dst, edge_weight)
    key = ("A", GT, NMM, tuple(tuple(m) for mm in vt_mms for m in mm))
    if key not in _cache:
        _cache[key] = _build_launch_a(vt_mms, GT, NMM)
    ncA = _cache[key]
    if "B" not in _cache:
        _cache["B"] = _build_launch_b()
    ncB = _cache["B"]

    wf = _fold_weights(weight)
    swt = np.ascontiguousarray(sw.transpose(1, 0, 2)).reshape(P, NMM * GSZ).astype(BF)
    # wrapped int16 gather indices (see launch A): k = slot*128 + lane
    idx_flat = idx_np.reshape(GT * P).astype(np.int16)
    idx16 = np.ascontiguousarray(np.tile(idx_flat.reshape(-1, 16).T, (8, 1)))
    in_maps = []
    for b in range(B):
        in_maps.append({
            "xb": np.ascontiguousarray(x[b]).astype(BF),
            "idx": idx16, "swt": swt, "wf": wf,
        })
    resA = run_bass_kernel_spmd(ncA, in_maps, core_ids=list(range(B)))

    # host: combine BN stats across cores (equal counts -> simple average)
    st = np.stack([np.asarray(resA.results[b]["stats"]) for b in range(B)])  # [B, 128, 4]
    mean = st[:, :, [0, 2]].mean(0)                               # [128, 2]
    ex2 = st[:, :, [1, 3]].mean(0)
    var = ex2 - mean * mean
    g2 = gamma.reshape(2, P).T                                    # [128, 2]
    b2 = beta.reshape(2, P).T
    scale = (g2 / np.sqrt(var + EPS)).astype(np.float32)
    shift = (b2 - mean * scale).astype(np.float32)

    in_maps_b = [{"rawT": resA.results[b]["rawT"], "sc": scale, "sh": shift}
                 for b in range(B)]
    resB = run_bass_kernel_spmd(ncB, in_maps_b, core_ids=list(range(B)))
    out = np.stack([np.asarray(resB.results[b]["out"]).astype(np.float32) for b in range(B)])
    # bias cancels inside training-mode BN (shifts the mean only); gamma/beta applied above
    return out


# revision 14
# speedup vs baseline: 1.0074x; 1.0001x over previous
"""Trainium2 Bass kernel for nn_ConvBlock (Chebyshev graph conv + BatchNorm + ReLU).

Sharding: data-parallel over batch (B=8 -> 1 sample per NeuronCore).
Per core: power-basis Chebyshev recursion z_j = L z_{j-1} via batched
dma_gather row gathers (bf16) + PE scatter-matmuls (edge weights folded into
host-built one-hot scatter blocks, SBUF-resident), K-stacked GEMM with
host-folded power-basis weights consuming z^T via DMA-transpose loads,
BatchNorm stats on device (combined across cores on host between two
launches), normalize+ReLU+transpose on device. All matmul operands bf16
(fp32 PSUM accumulation).
"""
import os, sys
sys.path.insert(0, '/opt/trn_rl_repo')
import numpy as np
import ml_dtypes
from contextlib import ExitStack

import concourse.bass as bass
import concourse.tile as tile
from concourse import bacc, mybir
from concourse.bass_utils import run_bass_kernel_spmd

B, V, E = 8, 12288, 98304
FIN, FOUT, K = 256, 256, 4
EPS = 1e-5
P = 128
GSZ = 64            # dst-group node window (S_w block width)
NVT = V // P        # 96 vtiles (group pairs)
GB = 4              # vtiles per gather batch
NB = NVT // GB      # 24 batches
SUP = 2048          # nodes per GEMM super-chunk
NSC = V // SUP      # 6 super-chunks
NCH = 24            # 512-node chunks for BN stats granularity

F32 = mybir.dt.float32
BF16 = mybir.dt.bfloat16
AF = mybir.ActivationFunctionType
BF = ml_dtypes.bfloat16

_cache = {}


def _build_schedule(edge_src, edge_dst, edge_weight):
    """Group edges by 64-node dst windows; pack full 128-lane gather slots per
    window, then merge the two windows' remainders of each vtile into one
    shared gather slot when they fit (saves gather rows).

    Returns:
      idx_np   [GT, P]   gather src row per (slot, lane)
      sw       [NMM, P, GSZ] scatter blocks (one per matmul)
      vt_mms   per vtile: list of (gather_slot, sw_slot, h)
      GT, NMM
    """
    g_of_e = edge_dst // GSZ
    order = np.argsort(g_of_e, kind='stable')
    NG = V // GSZ
    counts = np.bincount(g_of_e, minlength=NG)
    pos = np.zeros(NG + 1, np.int64)
    np.cumsum(counts, out=pos[1:])
    edges_of_g = [order[pos[g]:pos[g + 1]] for g in range(NG)]

    idx_rows = []
    sw_blocks = []
    vt_mms = [[] for _ in range(NVT)]

    def add_slot(parts):
        """parts: list of (edge_ids, h). One gather slot, one sw block per part."""
        gt = len(idx_rows)
        lanes = np.zeros(P, np.int32)
        off = 0
        entries = []
        for eids, h in parts:
            n = len(eids)
            lanes[off:off + n] = edge_src[eids]
            blk = np.zeros((P, GSZ), np.float32)
            blk[np.arange(off, off + n), edge_dst[eids] % GSZ] = edge_weight[eids]
            st = len(sw_blocks)
            sw_blocks.append(blk)
            entries.append((gt, st, h))
            off += n
        idx_rows.append(lanes)
        return entries

    for vt in range(NVT):
        g0, g1 = 2 * vt, 2 * vt + 1
        rema = []
        for g, h in ((g0, 0), (g1, 1)):
            eg = edges_of_g[g]
            nfull = len(eg) // P
            for s in range(nfull):
                vt_mms[vt] += add_slot([(eg[s * P:(s + 1) * P], h)])
            rem = eg[nfull * P:]
            if len(rem):
                rema.append((rem, h))
        if rema:
            if len(rema) == 2 and len(rema[0][0]) + len(rema[1][0]) <= P:
                vt_mms[vt] += add_slot(rema)
            else:
                for rem, h in rema:
                    vt_mms[vt] += add_slot([(rem, h)])

    idx_np = np.stack(idx_rows)                    # [GT, P]
    sw = np.stack(sw_blocks)                       # [NMM, P, GSZ]
    return idx_np, sw, vt_mms, len(idx_rows), len(sw_blocks)


def _fold_weights(weight):
    # out = sum_k T_k(L) x W_k ; T0=I, T1=L, T2=2L^2-1, T3=4L^3-3L
    # power basis z_j = L^j x :  out = sum_j z_j Wf_j
    W = weight.astype(np.float64)
    Wf = np.stack([W[0] - W[2], W[1] - 3.0 * W[3], 2.0 * W[2], 4.0 * W[3]])
    return Wf.reshape(K * FIN, FOUT).reshape(8, P, FOUT).astype(BF)


def _batches(vt_mms, GT):
    """Per gather batch of GB vtiles: (g0, ng, sw0, [(vt, mms_rebased)...]).
    Gather slots and sw slots are both contiguous in vtile order."""
    out = []
    for b in range(NB):
        vts = list(range(b * GB, (b + 1) * GB))
        g0 = vt_mms[vts[0]][0][0]
        sw0 = vt_mms[vts[0]][0][1]
        gmax, smax = 0, 0
        ventries = []
        for vt in vts:
            mms = [(gt - g0, st - sw0, h) for (gt, st, h) in vt_mms[vt]]
            gmax = max(gmax, max(m[0] for m in mms) + 1)
            smax = max(smax, max(m[1] for m in mms) + 1)
            ventries.append((vt, mms))
        out.append((g0, gmax, sw0, smax, ventries))
    return out


def _build_launch_a(vt_mms, GT, NMM):
    batches = _batches(vt_mms, GT)
    nc = bacc.Bacc("TRN2", target_bir_lowering=False, debug=False, num_devices=8)
    xb = nc.dram_tensor("xb", [V, FIN], BF16, kind="ExternalInput").ap()
    # int16 gather indices, wrapped: idx k of the global gather-lane order
    # (k = slot*128 + lane) lives at [k % 16, k // 16], replicated x8 rows.
    idx = nc.dram_tensor("idx", [P, GT * 8], mybir.dt.int16, kind="ExternalInput").ap()
    swt = nc.dram_tensor("swt", [P, NMM * GSZ], BF16, kind="ExternalInput").ap()
    wf = nc.dram_tensor("wf", [8, P, FOUT], BF16, kind="ExternalInput").ap()
    rawT = nc.dram_tensor("rawT", [2, P, V], BF16, kind="ExternalOutput").ap()
    stats = nc.dram_tensor("stats", [P, 4], F32, kind="ExternalOutput").ap()
    zd = [xb] + [nc.dram_tensor(f"z{j}", [V, FIN], BF16).ap() for j in (1, 2, 3)]

    with tile.TileContext(nc) as tc, ExitStack() as ctx:
        cpool = ctx.enter_context(tc.tile_pool(name="const", bufs=1))
        wf_t = cpool.tile([P, 8 * FOUT], BF16, tag="wf")
        nc.sync.dma_start(wf_t[:].rearrange("p (k o) -> p k o", k=8), wf.transpose([1, 0, 2]))

        # ---- Chebyshev (power-basis) recursion: z_j = L z_{j-1} ----
        with ExitStack() as rctx:
            rpool = rctx.enter_context(tc.tile_pool(name="rconst", bufs=1))
            idx_t = rpool.tile([P, GT * 8], mybir.dt.int16, tag="idx")
            nc.sync.dma_start(idx_t[:], idx[:, :])
            sw_t = rpool.tile([P, NMM * GSZ], BF16, tag="sw")
            # chunked resident load (pipelines with the first gathers)
            for (g0, ng, sw0, ns, _) in batches:
                nc.sync.dma_start(sw_t[:, sw0 * GSZ:(sw0 + ns) * GSZ],
                                  swt[:, sw0 * GSZ:(sw0 + ns) * GSZ])
            msgp = rctx.enter_context(tc.tile_pool(name="msgp", bufs=3))
            outp = rctx.enter_context(tc.tile_pool(name="outp", bufs=3))
            psp = rctx.enter_context(tc.tile_pool(name="psp", bufs=8, space="PSUM"))
            for j in (1, 2, 3):
                for (g0, ng, sw0, ns, ventries) in batches:
                    msg_t = msgp.tile([P, ng * FIN], BF16, tag="msg")
                    nc.gpsimd.dma_gather(
                        out_ap=msg_t[:].rearrange("p (t f) -> p t f", t=ng),
                        in_ap=zd[j - 1][:, :],
                        idxs_ap=idx_t[:, g0 * 8:(g0 + ng) * 8],
                        num_idxs=ng * P,
                        num_idxs_reg=ng * P,
                        elem_size=FIN,
                        single_packet=False)
                    zo = outp.tile([P, GB * FIN], BF16, tag="zo")
                    for i, (vt, mms) in enumerate(ventries):
                        ps = psp.tile([P, FIN], F32, tag="acc")
                        half_count = [sum(1 for m in mms if m[2] == hh) for hh in (0, 1)]
                        seen = [0, 0]
                        for (gt, st, h) in mms:
                            nc.tensor.matmul(
                                ps[h * GSZ:(h + 1) * GSZ, :],
                                sw_t[:, (sw0 + st) * GSZ:(sw0 + st + 1) * GSZ],
                                msg_t[:, gt * FIN:(gt + 1) * FIN],
                                start=(seen[h] == 0), stop=(seen[h] == half_count[h] - 1))
                            seen[h] += 1
                        if i % 2 == 0:
                            nc.scalar.activation(zo[:, i * FIN:(i + 1) * FIN], ps[:], AF.Copy)
                        else:
                            nc.vector.tensor_copy(zo[:, i * FIN:(i + 1) * FIN], ps[:])
                    b = ventries[0][0] // GB
                    nc.scalar.dma_start(
                        zd[j].rearrange("(b v p) f -> b p v f", v=GB, p=P)[b],
                        zo[:].rearrange("p (v f) -> p v f", v=GB))

        # ---- GEMM (zT via DMA-transpose, kt-outer so MMs trail each dmaT) ----
        with ExitStack() as gctx:
            ztp = gctx.enter_context(tc.tile_pool(name="ztp", bufs=4))
            rawp = gctx.enter_context(tc.tile_pool(name="rawp", bufs=2))
            psG = gctx.enter_context(tc.tile_pool(name="psG", bufs=8, space="PSUM"))
            sb = gctx.enter_context(tc.tile_pool(name="sb", bufs=1))
            stats_sb = sb.tile([P, 2 * NCH * 6], F32, tag="stats")
            for sc in range(NSC):
                zt = ztp.tile([P, 8 * SUP], BF16, tag="zt")
                pgs = [psG.tile([P, 512], F32, tag="pg", name=f"pg{sc}_{i}")
                       for i in range(8)]
                for kt in range(8):           # kt = (j, fh)
                    j, fh = kt // 2, kt % 2
                    nc.sync.dma_start(
                        zt[:, kt * SUP:(kt + 1) * SUP],
                        zd[j][sc * SUP:(sc + 1) * SUP, fh * P:(fh + 1) * P],
                        transpose=True)
                    for cc in range(SUP // 512):
                        for oh in range(2):
                            nc.tensor.matmul(
                                pgs[cc * 2 + oh][:],
                                wf_t[:, kt * FOUT + oh * P: kt * FOUT + oh * P + P],
                                zt[:, kt * SUP + cc * 512: kt * SUP + (cc + 1) * 512],
                                start=(kt == 0), stop=(kt == 7))
                raw_t = rawp.tile([P, 2 * SUP], BF16, tag="raw")
                for cc in range(SUP // 512):
                    for oh in range(2):
                        pg = pgs[cc * 2 + oh]
                        c = sc * (SUP // 512) + cc
                        nc.vector.bn_stats(stats_sb[:, (oh * NCH + c) * 6:(oh * NCH + c) * 6 + 6], pg[:])
                        if (cc + oh) % 2 == 0:
                            nc.scalar.activation(raw_t[:, oh * SUP + cc * 512: oh * SUP + (cc + 1) * 512], pg[:], AF.Copy)
                        else:
                            nc.vector.tensor_copy(raw_t[:, oh * SUP + cc * 512: oh * SUP + (cc + 1) * 512], pg[:])
                for oh in range(2):
                    nc.scalar.dma_start(rawT[oh][:, sc * SUP:(sc + 1) * SUP],
                                        raw_t[:, oh * SUP:(oh + 1) * SUP])
            aggr = sb.tile([P, 4], F32, tag="aggr")
            for oh in range(2):
                nc.vector.bn_aggr(aggr[:, oh * 2:oh * 2 + 2],
                                  stats_sb[:, oh * NCH * 6:(oh + 1) * NCH * 6])
            # stats out: [mean_h0, ex2_h0, mean_h1, ex2_h1]
            so = sb.tile([P, 4], F32, tag="so")
            for oh in range(2):
                m = aggr[:, oh * 2:oh * 2 + 1]
                v_ = aggr[:, oh * 2 + 1:oh * 2 + 2]
                nc.vector.tensor_copy(so[:, oh * 2:oh * 2 + 1], m)
                nc.vector.tensor_tensor(out=so[:, oh * 2 + 1:oh * 2 + 2], in0=m, in1=m,
                                        op=mybir.AluOpType.mult)
                nc.vector.tensor_tensor(out=so[:, oh * 2 + 1:oh * 2 + 2],
                                        in0=so[:, oh * 2 + 1:oh * 2 + 2], in1=v_,
                                        op=mybir.AluOpType.add)
            nc.sync.dma_start(stats[:, :], so[:])
    nc.compile()
    return nc


def _build_launch_b():
    nc = bacc.Bacc("TRN2", target_bir_lowering=False, debug=False, num_devices=8)
    rawT = nc.dram_tensor("rawT", [2, P, V], BF16, kind="ExternalInput").ap()
    sc = nc.dram_tensor("sc", [P, 2], F32, kind="ExternalInput").ap()
    sh = nc.dram_tensor("sh", [P, 2], F32, kind="ExternalInput").ap()
    out = nc.dram_tensor("out", [V, FOUT], BF16, kind="ExternalOutput").ap()
    CH2 = 8           # vtiles per chunk
    NC2 = NVT // CH2  # 12 chunks
    from concourse.masks import make_identity
    with tile.TileContext(nc) as tc, ExitStack() as ctx:
        cpool = ctx.enter_context(tc.tile_pool(name="const", bufs=1))
        ident = cpool.tile([P, P], BF16, tag="id")
        make_identity(nc, ident[:])
        sc_t = cpool.tile([P, 2], F32, tag="sc")
        sh_t = cpool.tile([P, 2], F32, tag="sh")
        nc.sync.dma_start(sc_t[:], sc[:, :])
        nc.sync.dma_start(sh_t[:], sh[:, :])
        pool = ctx.enter_context(tc.tile_pool(name="sb", bufs=2))
        psp = ctx.enter_context(tc.tile_pool(name="ps", bufs=4, space="PSUM"))
        for c in range(NC2):
            nt = pool.tile([P, 2 * CH2 * P], BF16, tag="nt")
            for oh in range(2):
                nc.sync.dma_start(nt[:, oh * CH2 * P:(oh + 1) * CH2 * P],
                                  rawT[oh][:, c * CH2 * P:(c + 1) * CH2 * P])
            for oh in range(2):
                nc.scalar.activation(
                    nt[:, oh * CH2 * P:(oh + 1) * CH2 * P],
                    nt[:, oh * CH2 * P:(oh + 1) * CH2 * P],
                    AF.Relu, bias=sh_t[:, oh:oh + 1], scale=sc_t[:, oh:oh + 1])
            ot = pool.tile([P, CH2 * FOUT], BF16, tag="ot")
            for vt in range(CH2):
                pt = psp.tile([P, FOUT], BF16, tag="pt")
                for oh in range(2):
                    nc.tensor.transpose(
                        pt[:, oh * P:(oh + 1) * P],
                        nt[:, oh * CH2 * P + vt * P: oh * CH2 * P + (vt + 1) * P],
                        ident[:])
                if vt % 2 == 0:
                    nc.vector.tensor_copy(ot[:, vt * FOUT:(vt + 1) * FOUT], pt[:])
                else:
                    nc.scalar.activation(ot[:, vt * FOUT:(vt + 1) * FOUT], pt[:], AF.Copy)
            nc.scalar.dma_start(
                out.rearrange("(c vt p) f -> c p vt f", vt=CH2, p=P)[c],
                ot[:].rearrange("p (vt f) -> p vt f", vt=CH2))
    nc.compile()
    return nc


def kernel(x, edge_weight, weight, bias, gamma, beta, edge_src, edge_dst):
    x = np.asarray(x, np.float32)
    edge_weight = np.asarray(edge_weight, np.float32)
    weight = np.asarray(weight, np.float32)
    gamma = np.asarray(gamma, np.float32)
    beta = np.asarray(beta, np.float32)
    edge_src = np.asarray(edge_src, np.int32)
    edge_dst = np.asarray(edge_dst, np.int32)

    idx_np, sw, vt_mms, GT, NMM = _build_schedule(edge_src, edge_dst, edge_weight)
    key = ("A", GT, NMM, tuple(tuple(m) for mm in vt_mms for m in mm))
    if key not in _cache:
        _cache[key] = _build_launch_a(vt_mms, GT, NMM)
    ncA = _cache[key]
    if "B" not in _cache:
        _cache["B"] = _build_launch_b()
    ncB = _cache["B"]

    wf = _fold_weights(weight)
    swt = np.ascontiguousarray(sw.transpose(1, 0, 2)).reshape(P, NMM * GSZ).astype(BF)
    # wrapped int16 gather indices (see launch A): k = slot*128 + lane
    idx_flat = idx_np.reshape(GT * P).astype(np.int16)
    idx16 = np.ascontiguousarray(np.tile(idx_flat.reshape(-1, 16).T, (8, 1)))
    in_maps = []
    for b in range(B):
        in_maps.append({
            "xb": np.ascontiguousarray(x[b]).astype(BF),
            "idx": idx16, "swt": swt, "wf": wf,
        })
    resA = run_bass_kernel_spmd(ncA, in_maps, core_ids=list(range(B)))

    # host: combine BN stats across cores (equal counts -> simple average)
    st = np.stack([np.asarray(resA.results[b]["stats"]) for b in range(B)])  # [B, 128, 4]
    mean = st[:, :, [0, 2]].mean(0)                               # [128, 2]
    ex2 = st[:, :, [1, 3]].mean(0)
    var = ex2 - mean * mean
    g2 = gamma.reshape(2, P).T                                    # [128, 2]
    b2 = beta.reshape(2, P).T
    scale = (g2 / np.sqrt(var + EPS)).astype(np.float32)
    shift = (b2 - mean * scale).astype(np.float32)

    in_maps_b = [{"rawT": resA.results[b]["rawT"], "sc": scale, "sh": shift}
                 for b in range(B)]
    resB = run_bass_kernel_spmd(ncB, in_maps_b, core_ids=list(range(B)))
    out = np.stack([np.asarray(resB.results[b]["out"]).astype(np.float32) for b in range(B)])
    # bias cancels inside training-mode BN (shifts the mean only); gamma/beta applied above
    return out
